# revision 37
# baseline (speedup 1.0000x reference)
"""Trainium2 Bass kernel for nn_Attn_88725434401526 (sparse_attention).

Reference computation:
    delta[b,l,m] = sum_d self_delta[b,m,l,d]
    P[b,l,m]     = emb_table[1+l] . self_attn[b,m]
    out[b,l]     = sum_m P[b,l,m] * delta[b,l,m] * value_w[0,m]

Shapes: B=16, MAX_LEN(m)=100, LOC_MAX(l)=20000, EMB=256, D=2.
Output: [16, 20000] float32.

Strategy (8 NeuronCores, loc_max sharded -> 2500 candidates per core):
  - host pre-reduces the d-pair and ships delta2[(b,m), l] in fp16:
    8 MB/core stream instead of 32 MB fp32 -> DMA is no longer the
    roofline; the PE matmul stream is.
  - (b,m) = 1600 rows in 13 chunks of 128. P2[(b,m), l] = emb . attn
    via fp16 PE matmuls (K=EMB as 2x128), N=512 l-tiles, PSUM fp32.
    Per k-half one stationary load streams all 5 l-tiles into 5
    rotating PSUM banks (out of 6) so fills pipeline at ~262 ns/MM.
  - prod = P2 * delta2: l-tiles 0,1 via ACT PSUM->SBUF fp16 copy (bank
    frees without waiting on the d2 stream) + GPSIMD product; l-tiles
    2-4 DVE product directly from PSUM (1x).  The first 3 chunks run
    all-copy so the PE is fully decoupled from d2 during DMA ramp-up.
  - weighted m-reduction: second matmul with stationary ws (value_w
    scattered block) accumulated over the 13 chunks, batched every 4
    chunks.  l-tiles map to 4 tensor-engine column groups
    (tile_position) over a 2-bank accumulator: strip 32j+0:16 x bank
    window, so 4 reduce MMs overlap in the array (~4 ns apart) and
    each (bank, strip) has exactly one accumulation group.
  - drain: two engine copies -> one 128-partition fp16 DMA of the raw
    strip layout; the host unscrambles to [16, 20000] fp32.
  - ~10 zero warmup matmuls at t=0 keep the PE HAM clock at 2.4 GHz
    by the time real matmuls start.

kernel(**inputs) takes the FULL unsharded inputs (numpy, keyed as in
setup_inputs()) and returns the FULL [16, 20000] float32 output.
"""
import sys

if "/opt/trn_rl_repo" not in sys.path:
    sys.path.insert(0, "/opt/trn_rl_repo")

import numpy as np
import ml_dtypes
import concourse.bass as bass
import concourse.mybir as mybir
from concourse import tile
from concourse.bass_utils import run_bass_kernel_spmd

FP32 = mybir.dt.float32
FP16 = mybir.dt.float16

B = 16
M = 100
LOC = 20000
EMB = 256
NCORES = 8
LCORE = LOC // NCORES          # 2500 candidates per core
G = B * M                      # 1600 (b,m) rows
P = 128
NCHUNK = (G + P - 1) // P      # 13 row chunks; last has 64 rows
LOFFS = [0, 512, 1024, 1536, 2048]
LWIDTH = [512, 512, 512, 512, 452]
NLT = len(LOFFS)
# l-tile -> (array col group, psum column window).  The out accumulator
# is 2 PSUM banks: tiles 0-3 live on partition strips 32j+0:16 of bank
# 0's column window, tile 4 on strip 0 of bank 1.  Each (bank, strip)
# pair hosts exactly one accumulation group across all chunks.
LGRP = [0, 1, 2, 3, 0]
LPSCOL = [0, 0, 0, 0, 512]
NWARM = 10

# per-(chunk, l-tile) engine assignment:
#   'G' = ACT copies PSUM->SBUF fp16 (frees the bank without waiting on
#         the d2 stream), GPSIMD prod from the copy
#   'A' = ACT copy, DVE prod from the copy
#   'V' = DVE copy, DVE prod from the copy
#   'D' = DVE prod directly from PSUM (1x, no copy)
# The first chunks run all-copy so the PE pipeline is decoupled from the
# d2 stream while the DMA queues ramp up.
LPLAN_EARLY = ["G", "G", "A", "V", "V"]
LPLAN_STEADY = ["G", "A", "A", "D", "D"]
NEARLY = 5


def _split_multi_waits(nc, maxw=1):
    """walrus codegen rejects >1 semaphore wait per instruction; split
    extra waits onto preceding NOPs on the same engine."""
    for fn in nc.m.functions:
        for bb in fn.blocks:
            newl = []
            for inst in bb.instructions:
                si = inst.sync_info
                if si is not None and si.on_wait and len(si.on_wait) > maxw:
                    waits = list(si.on_wait)
                    head, tail = waits[:-maxw], waits[-maxw:]
                    for i0 in range(0, len(head), maxw):
                        newl.append(
                            mybir.InstNoOp(
                                name=f"I-waitsplit-{nc.next_id()}",
                                engine=inst.engine,
                                sync_info=mybir.SyncInfo(
                                    on_wait=list(head[i0 : i0 + maxw]),
                                    on_update=[],
                                ),
                            )
                        )
                    inst.sync_info = mybir.SyncInfo(
                        on_wait=list(tail), on_update=list(si.on_update)
                    )
                newl.append(inst)
            bb.instructions = newl


def build_nc():
    nc = bass.Bass()
    d2 = nc.declare_dram_parameter("d2", [G, LCORE], FP16, isOutput=False)
    embT = nc.declare_dram_parameter("embT", [2, P, LCORE], FP16, isOutput=False)
    attnT = nc.declare_dram_parameter("attnT", [2, P, G], FP16, isOutput=False)
    ws = nc.declare_dram_parameter("ws", [P, NCHUNK * B], FP16, isOutput=False)
    # raw accumulator layout [128, 1024]: strip 32j+0:16 of column window
    # LPSCOL[li] holds l-tile li; the host unscrambles to [16, 20000].
    # fp16 transport (|out| <= ~10, quantization ~5e-4 rel, well inside
    # the accuracy budget) halves the final DMA.
    out = nc.declare_dram_parameter("out", [P, 1024], FP16, isOutput=True)

    with tile.TileContext(nc) as tc:
        with (
            tc.tile_pool(name="const", bufs=1) as cpool,
            tc.tile_pool(name="d2p", bufs=NCHUNK) as d2pool,
            tc.tile_pool(name="p2sbp", bufs=4) as p2sbpool,
            tc.tile_pool(name="prodp", bufs=7) as prodpool,
            tc.tile_pool(name="outp", bufs=1) as outpool,
            tc.tile_pool(name="ps", bufs=6, space="PSUM") as pspool,
            tc.tile_pool(name="pso", bufs=1, space="PSUM") as psopool,
        ):
            # --- warmup: zero matmuls to pull the PE HAM to 2.4 GHz ---
            zt = cpool.tile([P, 16], FP16)
            zm = cpool.tile([P, 512], FP16)
            nc.vector.memset(zt[:], 0.0)
            nc.vector.memset(zm[:], 0.0)
            for _ in range(NWARM):
                wps = pspool.tile([P, 512], FP32, tag="p2")
                nc.tensor.matmul(wps[:16, :], zt[:], zm[:], start=True, stop=True)

            # --- DMAs: constants first, then the whole d2 stream ---
            attnT_t = cpool.tile([P, 2, G], FP16)
            embT_t = cpool.tile([P, 2, LCORE], FP16)
            ws_t = cpool.tile([P, NCHUNK * B], FP16)
            # descriptor generation is ~600ns per dma_start and serial per
            # engine; split it across both HWDGE paths (sync + scalar) so
            # the d2 stream's descriptors issue ~3us earlier.
            nc.sync.dma_start(attnT_t[:, 0, :], attnT[0, :, :])
            nc.sync.dma_start(embT_t[:, 0, :], embT[0, :, :])
            nc.scalar.dma_start(attnT_t[:, 1, :], attnT[1, :, :])
            nc.scalar.dma_start(embT_t[:, 1, :], embT[1, :, :])
            nc.scalar.dma_start(ws_t[:], ws[:, :])
            d2_tiles = []
            for p in range(NCHUNK):
                g0 = p * P
                rows = min(P, G - g0)
                t = d2pool.tile([P, LCORE], FP16, name=f"d2t{p}", tag="d2")
                nc.sync.dma_start(t[:rows, :], d2[g0 : g0 + rows, :])
                d2_tiles.append(t)

            out_ps = psopool.tile([P, 1024], FP32)
            out_sb = outpool.tile([P, 1024], FP16)
            # initialize the gaps between partition strips so the final
            # whole-tile copies read defined data
            nc.vector.memset(out_ps[:], 0.0)

            prod_tiles = {}

            def front(p):
                g0 = p * P
                rows = min(P, G - g0)
                pss = [
                    pspool.tile([P, 512], FP32, name=f"ps{li}", tag="p2")
                    for li in range(NLT)
                ]
                # one LDWEIGHTS per k-half: 5 matmuls stream off the same
                # stationary (followers get ldweights=False post-pass)
                for k in range(2):
                    for li, ps in enumerate(pss):
                        l0, lw = LOFFS[li], LWIDTH[li]
                        nc.tensor.matmul(
                            ps[:rows, :lw],
                            attnT_t[:, k, g0 : g0 + rows],
                            embT_t[:, k, l0 : l0 + lw],
                            start=(k == 0),
                            stop=(k == 1),
                        )
                lplan = LPLAN_EARLY if p < NEARLY else LPLAN_STEADY
                for li, ps in enumerate(pss):
                    l0, lw = LOFFS[li], LWIDTH[li]
                    plan = lplan[li]
                    prod_t = prodpool.tile([P, 512], FP16, tag=f"pr{li}")
                    if plan == "D":
                        nc.vector.tensor_tensor(
                            prod_t[:rows, :lw],
                            ps[:rows, :lw],
                            d2_tiles[p][:rows, l0 : l0 + lw],
                            mybir.AluOpType.mult,
                        )
                    else:
                        p2sb = p2sbpool.tile([P, 512], FP16, tag=f"cp{li}")
                        if plan == "V":
                            nc.vector.tensor_copy(p2sb[:rows, :lw], ps[:rows, :lw])
                        else:
                            nc.scalar.copy(p2sb[:rows, :lw], ps[:rows, :lw])
                        eng = nc.gpsimd if plan == "G" else nc.vector
                        eng.tensor_tensor(
                            prod_t[:rows, :lw],
                            p2sb[:rows, :lw],
                            d2_tiles[p][:rows, l0 : l0 + lw],
                            mybir.AluOpType.mult,
                        )
                    prod_tiles[(p, li)] = prod_t

            def reduce(p):
                g0 = p * P
                rows = min(P, G - g0)
                for li in range(NLT):
                    lw = LWIDTH[li]
                    j, c0 = LGRP[li], LPSCOL[li]
                    nc.tensor.matmul(
                        out_ps[32 * j : 32 * j + B, c0 : c0 + lw],
                        ws_t[:rows, p * B : (p + 1) * B],
                        prod_tiles.pop((p, li))[:rows, :lw],
                        start=(p == 0),
                        stop=(p == NCHUNK - 1),
                        tile_position=(0, 32 * j),
                        skip_group_check=True,
                    )

            # reduce batches trail the front by >=4 chunks; the first
            # batch is kept small so it never waits on the GPSIMD
            # product queue, which itself trails the d2 DMA stream.
            RSCHED = {4: range(0, 2), 8: range(2, 7), 12: range(7, 11)}
            for p in range(NCHUNK):
                front(p)
                for q in RSCHED.get(p, ()):
                    reduce(q)
            for q in range(11, NCHUNK):
                reduce(q)

            # --- drain: raw accumulator -> SBUF -> two 128-partition DMAs
            # on separate HWDGE engines; the host unscrambles the layout ---
            nc.scalar.copy(out_sb[:, 0:512], out_ps[:, 0:512])
            nc.scalar.dma_start(out[:, 0:512], out_sb[:, 0:512])
            nc.vector.tensor_copy(out_sb[:, 512:1024], out_ps[:, 512:1024])
            nc.sync.dma_start(out[:, 512:1024], out_sb[:, 512:1024])

    _split_multi_waits(nc)
    return nc


_NC_CACHE = None


def _get_nc():
    global _NC_CACHE
    if _NC_CACHE is None:
        _NC_CACHE = build_nc()
    return _NC_CACHE


def make_in_maps(self_attn, self_delta, emb_table, value_w):
    self_attn = np.ascontiguousarray(self_attn, dtype=np.float32)
    emb_table = np.ascontiguousarray(emb_table, dtype=np.float32)
    value_w = np.ascontiguousarray(value_w, dtype=np.float32)
    f16 = ml_dtypes.float16 if hasattr(ml_dtypes, "float16") else np.float16

    # delta2[(b,m), l] = sum_d self_delta[b,m,l,d], fp16
    d2_full = np.asarray(self_delta, dtype=np.float32).sum(axis=-1)
    d2_full = d2_full.reshape(G, LOC).astype(f16)

    # attnT: [2, 128, 1600] = self_attn reshaped [(b,m), e], transposed
    attnT = (
        np.ascontiguousarray(self_attn.reshape(G, EMB).T)
        .reshape(2, P, G)
        .astype(f16)
    )

    # ws block matrix [128, 13*16]; ws[r, p*16+b] = w[m] for g=128p+r
    w = value_w[0]
    wseg = np.zeros((NCHUNK, P, B), np.float32)
    g = np.arange(G)
    wseg[g // P, g % P, g // M] = w[g % M]
    wseg = np.ascontiguousarray(
        wseg.transpose(1, 0, 2).reshape(P, NCHUNK * B)
    ).astype(f16)

    embT_all = np.ascontiguousarray(emb_table[1 : LOC + 1].T)  # [256, 20000]

    in_maps = []
    for c in range(NCORES):
        l0 = c * LCORE
        d2_c = np.ascontiguousarray(d2_full[:, l0 : l0 + LCORE])
        embT_c = (
            np.ascontiguousarray(embT_all[:, l0 : l0 + LCORE])
            .reshape(2, P, LCORE)
            .astype(f16)
        )
        in_maps.append(
            {"d2": d2_c, "embT": embT_c, "attnT": attnT, "ws": wseg}
        )
    return in_maps


def kernel(self_attn, self_delta, traj_len, emb_table, value_w, **_ignored):
    nc = _get_nc()
    in_maps = make_in_maps(self_attn, self_delta, emb_table, value_w)
    res = run_bass_kernel_spmd(nc, in_maps, list(range(NCORES)))
    out = np.empty((B, LOC), np.float32)
    for c in range(NCORES):
        raw = np.asarray(res.results[c]["out"]).astype(np.float32)
        for li in range(NLT):
            j, c0, l0, lw = LGRP[li], LPSCOL[li], LOFFS[li], LWIDTH[li]
            out[:, c * LCORE + l0 : c * LCORE + l0 + lw] = raw[
                32 * j : 32 * j + B, c0 : c0 + lw
            ]
    return out
